# revision 11
# baseline (speedup 1.0000x reference)
"""Trainium2 Bass kernel for a Mamba-style SSM single step.

Reference math (fp32):
    delta = softplus(x @ W_delta @ W_dt + b_dt)        [U, D]
    B = x @ W_B ; C = x @ W_C                          [U, N]
    abar = exp(delta[:,:,None] * A[None,:,:])          [U, D, N]
    h_new = abar * h + (delta*x)[:,:,None] * B[:,None,:]
    y = einsum('udn,un->ud', h_new, C) + D_vec * x

Distribution: tensor-parallel over d_inner across 8 cores (1280 each).
Each core redundantly computes the small projections (t = x@W_delta,
B, C) from full x^T / W_delta, then its own d-shard of the state
update. Layout on-chip: partition dim = users (U=128), free = (d, n).
"""

import os
import numpy as np

U, D_IN, RANK, N = 128, 10240, 320, 32
NCORES = 8
DSH = D_IN // NCORES            # 1280 per-core d shard
DT = int(os.environ.get("MAMBA_DT", "160"))   # d-tile size
NT = DSH // DT                  # tiles per core
CH = D_IN // 128                # contraction chunks for projections

_cache = {}


def _build_module():
    import concourse.bass as bass
    import concourse.mybir as mybir
    import concourse.tile as tile
    from concourse import bacc
    from contextlib import ExitStack

    fp32 = mybir.dt.float32
    AF = mybir.ActivationFunctionType
    OP = mybir.AluOpType

    nc = bacc.Bacc(
        "TRN2",
        target_bir_lowering=False,
        debug=False,
        enable_asserts=False,
        num_devices=NCORES,
    )

    # ---- DRAM I/O (per-core) ----
    use_f32r = os.environ.get("MAMBA_F32R", "1") == "1"
    mmdt = mybir.dt.float32r if use_f32r else fp32
    h_d = nc.dram_tensor("h_in", [U, DSH, N], fp32, kind="ExternalInput").ap()
    x_d = nc.dram_tensor("x_sh", [U, DSH], fp32, kind="ExternalInput").ap()
    xt_d = nc.dram_tensor("xT", [D_IN, U], mmdt, kind="ExternalInput").ap()
    wall_d = nc.dram_tensor("wall", [D_IN, RANK + 2 * N], mmdt, kind="ExternalInput").ap()
    wdt_d = nc.dram_tensor("wdt_aug", [RANK + 1, DSH], fp32, kind="ExternalInput").ap()
    a_d = nc.dram_tensor("a_sh", [DSH, N], fp32, kind="ExternalInput").ap()
    dbc_d = nc.dram_tensor("dbc", [U, DSH], fp32, kind="ExternalInput").ap()
    ident_d = nc.dram_tensor("ident", [128, 128], fp32, kind="ExternalInput").ap()
    hnew_d = nc.dram_tensor("h_out", [U, DSH, N], fp32, kind="ExternalOutput").ap()
    y_d = nc.dram_tensor("y_out", [U, DSH], fp32, kind="ExternalOutput").ap()

    with tile.TileContext(nc) as tc, ExitStack() as ctx:
        const = ctx.enter_context(tc.tile_pool(name="const", bufs=1))
        wpool = ctx.enter_context(tc.tile_pool(name="w", bufs=3))
        ppool = ctx.enter_context(tc.tile_pool(name="ps", bufs=2, space="PSUM"))
        hpool = ctx.enter_context(tc.tile_pool(name="h", bufs=2))
        apool = ctx.enter_context(tc.tile_pool(name="abc", bufs=2))

        # ---------------- Phase P: projections ----------------
        x_sb = const.tile([U, DSH], fp32, tag="x")
        nc.sync.dma_start(x_sb[:], x_d)
        dbc_sb = const.tile([U, DSH], fp32, tag="dbc")
        nc.sync.dma_start(dbc_sb[:], dbc_d)
        ident = const.tile([128, 128], fp32, tag="ident")
        nc.sync.dma_start(ident[:], ident_d)
        # W_dt_aug rows as three partition-chunks (128/128/65)
        wdt0 = const.tile([128, DSH], fp32, tag="wdt0")
        nc.sync.dma_start(wdt0[:], wdt_d[0:128, :])
        wdt1 = const.tile([128, DSH], fp32, tag="wdt1")
        nc.sync.dma_start(wdt1[:], wdt_d[128:256, :])
        wdt2 = const.tile([65, DSH], fp32, tag="wdt2")
        nc.sync.dma_start(wdt2[:], wdt_d[256:321, :])

        # t|B|C = x @ [W_delta | W_B | W_C]  (accumulate over 80 chunks)
        # float32r runs the PE at 1 cycle/row (vs 4 for fp32) for moving
        # free >= 256.
        tbc_ps = ppool.tile([128, RANK + 2 * N], fp32, tag="tbc")
        for c in range(CH):
            xt_c = wpool.tile([128, U], mmdt, tag="xt")
            nc.sync.dma_start(xt_c[:], xt_d[c * 128:(c + 1) * 128, :])
            w_c = wpool.tile([128, RANK + 2 * N], mmdt, tag="wall")
            nc.sync.dma_start(w_c[:], wall_d[c * 128:(c + 1) * 128, :])
            nc.tensor.matmul(
                tbc_ps[:], lhsT=xt_c[:], rhs=w_c[:],
                start=(c == 0), stop=(c == CH - 1),
            )
        t_sb = const.tile([128, RANK], fp32, tag="t")
        nc.scalar.copy(t_sb[:], tbc_ps[:, 0:RANK])
        bc_sb = const.tile([128, 2 * N], fp32, tag="bc")
        nc.scalar.copy(bc_sb[:], tbc_ps[:, RANK:RANK + 2 * N])

        # tT chunks (128/128/64 rows) + ones row for the bias
        tT0 = const.tile([128, U], fp32, tag="tT0")
        tT1 = const.tile([128, U], fp32, tag="tT1")
        tT2 = const.tile([65, U], fp32, tag="tT2")
        for j, (r0, rc, dst) in enumerate([(0, 128, tT0), (128, 128, tT1), (256, 64, tT2)]):
            tt_ps = ppool.tile([rc, 128], fp32, tag="ttps")
            nc.tensor.transpose(tt_ps[:], t_sb[:, r0:r0 + rc], ident[:])
            nc.scalar.copy(dst[0:rc, :], tt_ps[:])
        nc.vector.memset(tT2[64:65, :], 1.0)

        # delta = softplus(tT.T @ W_dt_aug)  per d-tile
        delta_sb = const.tile([U, DSH], fp32, tag="delta")
        for i in range(NT):
            sl = slice(i * DT, (i + 1) * DT)
            d_ps = ppool.tile([U, DT], fp32, tag="dps")
            nc.tensor.matmul(d_ps[:], lhsT=tT0[:], rhs=wdt0[:, sl], start=True, stop=False)
            nc.tensor.matmul(d_ps[:], lhsT=tT1[:], rhs=wdt1[:, sl], start=False, stop=False)
            nc.tensor.matmul(d_ps[:], lhsT=tT2[:], rhs=wdt2[:, sl], start=False, stop=True)
            # softplus(z) = ln(exp(z) + 1) — Exp and Ln share one ACT table
            nc.scalar.activation(d_ps[:], d_ps[:], AF.Exp)
            nc.scalar.activation(delta_sb[:, sl], d_ps[:], AF.Ln, bias=1.0)

        # dx = delta * x ; y init = D * x
        dx_sb = const.tile([U, DSH], fp32, tag="dx")
        nc.vector.tensor_tensor(dx_sb[:], delta_sb[:], x_sb[:], op=OP.mult)
        y_sb = const.tile([U, DSH], fp32, tag="y")
        nc.vector.tensor_tensor(y_sb[:], x_sb[:], dbc_sb[:], op=OP.mult)

        # ---------------- Phase E: state update ----------------
        # A staged once per tile into one partition, then broadcast
        # SBUF->SBUF (keeps the replicated read off HBM).
        B_view = bc_sb[:, 0:N].unsqueeze(1).broadcast_to([U, DT, N])
        C_view = bc_sb[:, N:2 * N].unsqueeze(1).broadcast_to([U, DT, N])
        for i in range(NT):
            sl = slice(i * DT, (i + 1) * DT)
            abc = apool.tile([U, DT, N], fp32, tag="abc")
            a_src = a_d[sl, :].unsqueeze(0).broadcast_to([U, DT, N])
            nc.sync.dma_start(abc[:], a_src)
            # h tile
            ht = hpool.tile([U, DT, N], fp32, tag="ht")
            nc.sync.dma_start(ht[:], h_d[:, sl, :])
            # tmp = delta (x) A   (in place over abc)
            dview = delta_sb[:, sl].unsqueeze(2).broadcast_to([U, DT, N])
            nc.vector.tensor_tensor(abc[:], dview, abc[:], op=OP.mult)
            # abar = exp(tmp)  (in place)
            nc.scalar.activation(abc[:], abc[:], AF.Exp)
            # ah = abar * h    (in place over ht)
            nc.vector.tensor_tensor(ht[:], abc[:], ht[:], op=OP.mult)
            # bx = dx (x) B  (into abc, abar no longer needed)
            dxview = dx_sb[:, sl].unsqueeze(2).broadcast_to([U, DT, N])
            nc.vector.tensor_tensor(abc[:], dxview, B_view, op=OP.mult)
            # h_new = ah + bx
            if os.environ.get("MAMBA_DMA_ADD", "0") == "1":
                nc.gpsimd.dma_start(ht[:], abc[:], accum_op=OP.add)
            else:
                nc.vector.tensor_tensor(ht[:], abc[:], ht[:], op=OP.add)
            # q = h_new * C  (into abc)
            nc.vector.tensor_tensor(abc[:], ht[:], C_view, op=OP.mult)
            # y[:, sl] += sum_n q
            yp = wpool.tile([U, DT], fp32, tag="yp")
            nc.vector.tensor_reduce(yp[:], abc[:], axis=mybir.AxisListType.X, op=OP.add)
            nc.vector.tensor_tensor(y_sb[:, sl], yp[:], y_sb[:, sl], op=OP.add)
            nc.sync.dma_start(hnew_d[:, sl, :], ht[:])
        nc.sync.dma_start(y_d, y_sb[:])

    nc.compile()
    return nc


def _get_module():
    if "nc" not in _cache:
        _cache["nc"] = _build_module()
    return _cache["nc"]


def _make_in_maps(x, h, W_delta, W_dt, b_dt, W_B, W_C, A, D):
    x = np.ascontiguousarray(np.asarray(x, np.float32))
    h = np.asarray(h, np.float32)
    wall = np.ascontiguousarray(
        np.concatenate(
            [np.asarray(W_delta, np.float32), np.asarray(W_B, np.float32),
             np.asarray(W_C, np.float32)], axis=1)
    )
    xt = np.ascontiguousarray(x.T)
    wdt_aug = np.ascontiguousarray(
        np.concatenate(
            [np.asarray(W_dt, np.float32),
             np.asarray(b_dt, np.float32)[None, :]], axis=0)
    )
    A = np.asarray(A, np.float32)
    D = np.asarray(D, np.float32)
    ident = np.eye(128, dtype=np.float32)
    in_maps = []
    for k in range(NCORES):
        sl = slice(k * DSH, (k + 1) * DSH)
        in_maps.append({
            "h_in": np.ascontiguousarray(h[:, sl, :]),
            "x_sh": np.ascontiguousarray(x[:, sl]),
            "xT": xt,
            "wall": wall,
            "wdt_aug": np.ascontiguousarray(wdt_aug[:, sl]),
            "a_sh": np.ascontiguousarray(A[sl, :]),
            "dbc": np.ascontiguousarray(
                np.broadcast_to(D[sl][None, :], (U, DSH))),
            "ident": ident,
        })
    return in_maps


def _run(in_maps, trace=False):
    from concourse import bass_utils
    nc = _get_module()
    res = bass_utils.run_bass_kernel_spmd(
        nc, in_maps, core_ids=list(range(NCORES)), trace=trace,
    )
    return res


def _gather(results):
    y = np.concatenate([results[k]["y_out"] for k in range(NCORES)], axis=1)
    h_new = np.concatenate([results[k]["h_out"] for k in range(NCORES)], axis=1)
    return y.astype(np.float32), h_new.astype(np.float32)


def kernel(x, h, W_delta, W_dt, b_dt, W_B, W_C, A, D):
    in_maps = _make_in_maps(x, h, W_delta, W_dt, b_dt, W_B, W_C, A, D)
    res = _run(in_maps, trace=False)
    return _gather(res.results)


def _install_ntff_hook():
    """Shim antenv.axon_hooks (absent in this image) and register the
    ctypes NTFF profile hook so trace=True yields exec_time_ns."""
    import sys
    import types
    if "antenv.axon_hooks" not in sys.modules:
        import antenv
        mod = types.ModuleType("antenv.axon_hooks")
        mod._hook = None

        def set_axon_ntff_profile_hook(h):
            mod._hook = h

        def get_axon_ntff_profile_hook():
            return mod._hook

        mod.set_axon_ntff_profile_hook = set_axon_ntff_profile_hook
        mod.get_axon_ntff_profile_hook = get_axon_ntff_profile_hook
        sys.modules["antenv.axon_hooks"] = mod
        antenv.axon_hooks = mod
    import antenv.axon_hooks as ah
    if ah._hook is None:
        from trn_agent_boot.trn_boot import _ntff_profile_via_ctypes
        hook = _ntff_profile_via_ctypes("/opt/axon/libaxon_pjrt.so")
        if hook is not None:
            ah.set_axon_ntff_profile_hook(hook)
    # avoid network-dependent artifact upload in the trace path
    from concourse import bass_utils
    bass_utils.upload_artifacts = lambda tmpdir: f"local:{tmpdir}"


def kernel_traced(x, h, W_delta, W_dt, b_dt, W_B, W_C, A, D):
    """Like kernel() but with NTFF tracing; returns ((y, h_new), results)."""
    _install_ntff_hook()
    in_maps = _make_in_maps(x, h, W_delta, W_dt, b_dt, W_B, W_C, A, D)
    res = _run(in_maps, trace=True)
    return _gather(res.results), res


# revision 13
# speedup vs baseline: 1.2394x; 1.2394x over previous
"""Trainium2 Bass kernel for a Mamba-style SSM single step.

Reference math (fp32):
    delta = softplus(x @ W_delta @ W_dt + b_dt)        [U, D]
    B = x @ W_B ; C = x @ W_C                          [U, N]
    abar = exp(delta[:,:,None] * A[None,:,:])          [U, D, N]
    h_new = abar * h + (delta*x)[:,:,None] * B[:,None,:]
    y = einsum('udn,un->ud', h_new, C) + D_vec * x

Distribution: tensor-parallel over d_inner across 8 cores (1280 each).
Each core redundantly computes the small projections (t = x@W_delta,
B, C) from full x^T / W_delta, then its own d-shard of the state
update. Layout on-chip: partition dim = users (U=128), free = (d, n).
"""

import os
import numpy as np

U, D_IN, RANK, N = 128, 10240, 320, 32
NCORES = 8
DSH = D_IN // NCORES            # 1280 per-core d shard
DT = int(os.environ.get("MAMBA_DT", "128"))   # d-tile size
NT = DSH // DT                  # tiles per core
CH = D_IN // 128                # contraction chunks for projections

_cache = {}


def _build_module():
    import concourse.bass as bass
    import concourse.mybir as mybir
    import concourse.tile as tile
    from concourse import bacc
    from contextlib import ExitStack

    fp32 = mybir.dt.float32
    AF = mybir.ActivationFunctionType
    OP = mybir.AluOpType

    nc = bacc.Bacc(
        "TRN2",
        target_bir_lowering=False,
        debug=False,
        enable_asserts=False,
        num_devices=NCORES,
    )

    # ---- DRAM I/O (per-core) ----
    use_f32r = os.environ.get("MAMBA_F32R", "1") == "1"
    mmdt = mybir.dt.float32r if use_f32r else fp32
    h_d = nc.dram_tensor("h_in", [U, DSH, N], fp32, kind="ExternalInput").ap()
    x_d = nc.dram_tensor("x_sh", [U, DSH], fp32, kind="ExternalInput").ap()
    xt_d = nc.dram_tensor("xT", [D_IN, U], mmdt, kind="ExternalInput").ap()
    wall_d = nc.dram_tensor("wall", [D_IN, RANK + 2 * N], mmdt, kind="ExternalInput").ap()
    wdt_d = nc.dram_tensor("wdt_aug", [RANK + 1, DSH], fp32, kind="ExternalInput").ap()
    a_d = nc.dram_tensor("a_sh", [DSH, N], fp32, kind="ExternalInput").ap()
    dbc_d = nc.dram_tensor("dbc", [U, DSH], fp32, kind="ExternalInput").ap()
    ident_d = nc.dram_tensor("ident", [128, 128], fp32, kind="ExternalInput").ap()
    hnew_d = nc.dram_tensor("h_out", [U, DSH, N], fp32, kind="ExternalOutput").ap()
    y_d = nc.dram_tensor("y_out", [U, DSH], fp32, kind="ExternalOutput").ap()

    with tile.TileContext(nc) as tc, ExitStack() as ctx:
        const = ctx.enter_context(tc.tile_pool(name="const", bufs=1))
        wpool = ctx.enter_context(tc.tile_pool(name="w", bufs=2))
        ppool = ctx.enter_context(tc.tile_pool(name="ps", bufs=2, space="PSUM"))
        hpool = ctx.enter_context(tc.tile_pool(name="h", bufs=2))
        apool = ctx.enter_context(tc.tile_pool(name="abc", bufs=2))

        # ---------------- Phase P: projections ----------------
        x_sb = const.tile([U, DSH], fp32, tag="x")
        nc.sync.dma_start(x_sb[:], x_d)
        dbc_sb = const.tile([U, DSH], fp32, tag="dbc")
        nc.sync.dma_start(dbc_sb[:], dbc_d)
        ident = const.tile([128, 128], fp32, tag="ident")
        nc.sync.dma_start(ident[:], ident_d)
        # W_dt_aug rows as three partition-chunks (128/128/65)
        wdt0 = const.tile([128, DSH], fp32, tag="wdt0")
        nc.sync.dma_start(wdt0[:], wdt_d[0:128, :])
        wdt1 = const.tile([128, DSH], fp32, tag="wdt1")
        nc.sync.dma_start(wdt1[:], wdt_d[128:256, :])
        wdt2 = const.tile([65, DSH], fp32, tag="wdt2")
        nc.sync.dma_start(wdt2[:], wdt_d[256:321, :])

        # t|B|C = x @ [W_delta | W_B | W_C]  (accumulate over 80 chunks)
        # float32r runs the PE at 1 cycle/row (vs 4 for fp32) for moving
        # free >= 256.
        tbc_ps = ppool.tile([128, RANK + 2 * N], fp32, tag="tbc")
        KG = 8  # contraction chunks per DMA batch
        W2 = RANK + 2 * N
        xt_g = xt_d.rearrange("(g k p) u -> g p k u", k=KG, p=128)
        wall_g = wall_d.rearrange("(g k p) w -> g p k w", k=KG, p=128)
        for g in range(CH // KG):
            xt_c = wpool.tile([128, KG, U], mmdt, tag="xt")
            nc.sync.dma_start(xt_c[:], xt_g[g])
            w_c = wpool.tile([128, KG, W2], mmdt, tag="wall")
            nc.sync.dma_start(w_c[:], wall_g[g])
            for k in range(KG):
                c = g * KG + k
                nc.tensor.matmul(
                    tbc_ps[:], lhsT=xt_c[:, k, :], rhs=w_c[:, k, :],
                    start=(c == 0), stop=(c == CH - 1),
                )
        t_sb = const.tile([128, RANK], fp32, tag="t")
        nc.scalar.copy(t_sb[:], tbc_ps[:, 0:RANK])
        bc_sb = const.tile([128, 2 * N], fp32, tag="bc")
        nc.scalar.copy(bc_sb[:], tbc_ps[:, RANK:RANK + 2 * N])

        # tT chunks (128/128/64 rows) + ones row for the bias
        tT0 = const.tile([128, U], fp32, tag="tT0")
        tT1 = const.tile([128, U], fp32, tag="tT1")
        tT2 = const.tile([65, U], fp32, tag="tT2")
        for j, (r0, rc, dst) in enumerate([(0, 128, tT0), (128, 128, tT1), (256, 64, tT2)]):
            tt_ps = ppool.tile([rc, 128], fp32, tag="ttps")
            nc.tensor.transpose(tt_ps[:], t_sb[:, r0:r0 + rc], ident[:])
            nc.scalar.copy(dst[0:rc, :], tt_ps[:])
        nc.vector.memset(tT2[64:65, :], 1.0)

        # delta = softplus(tT.T @ W_dt_aug)  per d-tile
        delta_sb = const.tile([U, DSH], fp32, tag="delta")
        for i in range(NT):
            sl = slice(i * DT, (i + 1) * DT)
            d_ps = ppool.tile([U, DT], fp32, tag="dps")
            nc.tensor.matmul(d_ps[:], lhsT=tT0[:], rhs=wdt0[:, sl], start=True, stop=False)
            nc.tensor.matmul(d_ps[:], lhsT=tT1[:], rhs=wdt1[:, sl], start=False, stop=False)
            nc.tensor.matmul(d_ps[:], lhsT=tT2[:], rhs=wdt2[:, sl], start=False, stop=True)
            # softplus(z) = ln(exp(z) + 1) — Exp and Ln share one ACT table
            nc.scalar.activation(d_ps[:], d_ps[:], AF.Exp)
            nc.scalar.activation(delta_sb[:, sl], d_ps[:], AF.Ln, bias=1.0)

        # dx = delta * x ; y init = D * x
        dx_sb = const.tile([U, DSH], fp32, tag="dx")
        nc.vector.tensor_tensor(dx_sb[:], delta_sb[:], x_sb[:], op=OP.mult)
        y_sb = const.tile([U, DSH], fp32, tag="y")
        nc.vector.tensor_tensor(y_sb[:], x_sb[:], dbc_sb[:], op=OP.mult)

        # ---------------- Phase E: state update ----------------
        # A staged once per tile into one partition, then broadcast
        # SBUF->SBUF (keeps the replicated read off HBM).
        B_view = bc_sb[:, 0:N].unsqueeze(1).broadcast_to([U, DT, N])
        C_view = bc_sb[:, N:2 * N].unsqueeze(1).broadcast_to([U, DT, N])
        for i in range(NT):
            sl = slice(i * DT, (i + 1) * DT)
            abc = apool.tile([U, DT, N], fp32, tag="abc")
            a_src = a_d[sl, :].unsqueeze(0).broadcast_to([U, DT, N])
            nc.sync.dma_start(abc[:], a_src)
            # h tile
            ht = hpool.tile([U, DT, N], fp32, tag="ht")
            nc.sync.dma_start(ht[:], h_d[:, sl, :])
            # tmp = delta (x) A   (in place over abc)
            dview = delta_sb[:, sl].unsqueeze(2).broadcast_to([U, DT, N])
            nc.vector.tensor_tensor(abc[:], dview, abc[:], op=OP.mult)
            # abar = exp(tmp)  (in place)
            nc.scalar.activation(abc[:], abc[:], AF.Exp)
            # ah = abar * h    (in place over ht)
            nc.vector.tensor_tensor(ht[:], abc[:], ht[:], op=OP.mult)
            # bx = dx (x) B  (into abc, abar no longer needed)
            dxview = dx_sb[:, sl].unsqueeze(2).broadcast_to([U, DT, N])
            nc.vector.tensor_tensor(abc[:], dxview, B_view, op=OP.mult)
            # h_new = ah + bx
            if os.environ.get("MAMBA_DMA_ADD", "0") == "1":
                nc.gpsimd.dma_start(ht[:], abc[:], accum_op=OP.add)
            else:
                nc.vector.tensor_tensor(ht[:], abc[:], ht[:], op=OP.add)
            # q = h_new * C  (into abc)
            nc.vector.tensor_tensor(abc[:], ht[:], C_view, op=OP.mult)
            # y[:, sl] += sum_n q
            yp = wpool.tile([U, DT], fp32, tag="yp")
            nc.vector.tensor_reduce(yp[:], abc[:], axis=mybir.AxisListType.X, op=OP.add)
            nc.vector.tensor_tensor(y_sb[:, sl], yp[:], y_sb[:, sl], op=OP.add)
            nc.sync.dma_start(hnew_d[:, sl, :], ht[:])
        nc.sync.dma_start(y_d, y_sb[:])

    nc.compile()
    return nc


def _get_module():
    if "nc" not in _cache:
        _cache["nc"] = _build_module()
    return _cache["nc"]


def _make_in_maps(x, h, W_delta, W_dt, b_dt, W_B, W_C, A, D):
    x = np.ascontiguousarray(np.asarray(x, np.float32))
    h = np.asarray(h, np.float32)
    wall = np.ascontiguousarray(
        np.concatenate(
            [np.asarray(W_delta, np.float32), np.asarray(W_B, np.float32),
             np.asarray(W_C, np.float32)], axis=1)
    )
    xt = np.ascontiguousarray(x.T)
    wdt_aug = np.ascontiguousarray(
        np.concatenate(
            [np.asarray(W_dt, np.float32),
             np.asarray(b_dt, np.float32)[None, :]], axis=0)
    )
    A = np.asarray(A, np.float32)
    D = np.asarray(D, np.float32)
    ident = np.eye(128, dtype=np.float32)
    in_maps = []
    for k in range(NCORES):
        sl = slice(k * DSH, (k + 1) * DSH)
        in_maps.append({
            "h_in": np.ascontiguousarray(h[:, sl, :]),
            "x_sh": np.ascontiguousarray(x[:, sl]),
            "xT": xt,
            "wall": wall,
            "wdt_aug": np.ascontiguousarray(wdt_aug[:, sl]),
            "a_sh": np.ascontiguousarray(A[sl, :]),
            "dbc": np.ascontiguousarray(
                np.broadcast_to(D[sl][None, :], (U, DSH))),
            "ident": ident,
        })
    return in_maps


def _run(in_maps, trace=False):
    from concourse import bass_utils
    nc = _get_module()
    res = bass_utils.run_bass_kernel_spmd(
        nc, in_maps, core_ids=list(range(NCORES)), trace=trace,
    )
    return res


def _gather(results):
    y = np.concatenate([results[k]["y_out"] for k in range(NCORES)], axis=1)
    h_new = np.concatenate([results[k]["h_out"] for k in range(NCORES)], axis=1)
    return y.astype(np.float32), h_new.astype(np.float32)


def kernel(x, h, W_delta, W_dt, b_dt, W_B, W_C, A, D):
    in_maps = _make_in_maps(x, h, W_delta, W_dt, b_dt, W_B, W_C, A, D)
    res = _run(in_maps, trace=False)
    return _gather(res.results)


def _install_ntff_hook():
    """Shim antenv.axon_hooks (absent in this image) and register the
    ctypes NTFF profile hook so trace=True yields exec_time_ns."""
    import sys
    import types
    if "antenv.axon_hooks" not in sys.modules:
        import antenv
        mod = types.ModuleType("antenv.axon_hooks")
        mod._hook = None

        def set_axon_ntff_profile_hook(h):
            mod._hook = h

        def get_axon_ntff_profile_hook():
            return mod._hook

        mod.set_axon_ntff_profile_hook = set_axon_ntff_profile_hook
        mod.get_axon_ntff_profile_hook = get_axon_ntff_profile_hook
        sys.modules["antenv.axon_hooks"] = mod
        antenv.axon_hooks = mod
    import antenv.axon_hooks as ah
    if ah._hook is None:
        from trn_agent_boot.trn_boot import _ntff_profile_via_ctypes
        hook = _ntff_profile_via_ctypes("/opt/axon/libaxon_pjrt.so")
        if hook is not None:
            ah.set_axon_ntff_profile_hook(hook)
    # avoid network-dependent artifact upload in the trace path
    from concourse import bass_utils
    bass_utils.upload_artifacts = lambda tmpdir: f"local:{tmpdir}"


def kernel_traced(x, h, W_delta, W_dt, b_dt, W_B, W_C, A, D):
    """Like kernel() but with NTFF tracing; returns ((y, h_new), results)."""
    _install_ntff_hook()
    in_maps = _make_in_maps(x, h, W_delta, W_dt, b_dt, W_B, W_C, A, D)
    res = _run(in_maps, trace=True)
    return _gather(res.results), res
